# revision 18
# baseline (speedup 1.0000x reference)
"""GCN layer (COO SpMM + linear) on 8 Trainium2 NeuronCores.

Strategy (per sharding hint): shard destination nodes across the 8 cores
(12,500 rows each); partition edges by destination so the segment-sum is
core-local; replicate X (each core gathers source rows from its own full
copy in HBM, stored bf16) and the small [128,128] weight.

Per-core kernel:
  - dest nodes are grouped into blocks of 128; blocks into super-blocks of
    SBLK=28 (4 block accumulators packed per PSUM bank -> 7 banks).
  - dma_gather uses int16 indices, so X is windowed into chunks of 32768
    rows. Edge slots are laid out per (super-block, chunk) run, block by
    block, with each (block, chunk) cell's slot quota rounded to 16 (max
    over the 8 cores, so one SPMD program serves all of them). Batches of
    128 slots may span block boundaries; the per-batch matmul is split
    into 16-aligned partition sub-ranges, one per touched block.
  - per batch (128 edge slots, one per SBUF partition):
      Xg   = dma_gather of bf16 X[col[e]] rows        [128e, 128f]
      S_T  = val[e] * (iota[d] == dloc[e])   (one fused DVE tensor_scalar,
             all-bf16 operands -> 4x DVE mode)
      h.T += Xg.T @ S_T   (bf16 PE matmul into the block's PSUM column)
  - per block: y = (h.T).T @ W.T + b via a second matmul, staged per
    super-block and DMA'd out in fp32.
"""

import sys

import numpy as np

sys.path.insert(0, "/opt/trn_rl_repo")

import concourse.bacc as bacc
import concourse.mybir as mybir
import concourse.tile as tile
from concourse.bass_utils import run_bass_kernel_spmd

N_NODES = 100000
D = 128
N_CORES = 8
NPC = N_NODES // N_CORES  # nodes per core
P = 128
CHUNK = 32768  # int16 index window over X rows
SBLK = 7  # blocks per super-block (one PSUM bank per block + 1 proj bank)
GB = 64  # batches per dma_gather instruction (ring entries ~ num_idxs/16)
QUANT = 32  # slot quantum per (block, chunk) cell (PE tile_position rule)
SCRATCH = 16384  # SWDGE ring: SCRATCH/16 = 1024 entries/queue (gather uses idxs/16+1)

F32 = mybir.dt.float32
BF16 = mybir.dt.bfloat16
I16 = mybir.dt.int16
NP_BF16 = mybir.dt.np(BF16)
XDT = BF16  # gather payload dtype (BF16 or F32; F32 only for experiments)


def _chunk_bounds(n_nodes, chunk):
    ch = list(range(0, n_nodes, chunk)) + [n_nodes]
    return np.array(ch, dtype=np.int64)


def _mm_pieces(lo, hi):
    """Split [lo, hi) into PE-legal (tile_position) partition ranges:
    size<=32 may start at 0/32/64/96, size<=64 at 0/64, larger at 0."""
    pieces = []
    while lo < hi:
        if lo == 0 or lo == 64 and hi - lo <= 64:
            pieces.append((lo, hi))
            break
        nxt = min(hi, lo + 32)
        pieces.append((lo, nxt))
        lo = nxt
    return pieces


def _schedule(counts, sblk):
    """counts: [n_cores, nb, nq] -> shared slot/segment schedule.

    Slots for cell (b, q) = max edge count over the 8 cores (exact, no
    rounding). Batches of 128 slots span blocks freely; each (batch, block)
    intersection is a SEGMENT with its own global index s. Every segment
    runs as a full-128-partition matmul whose one-hot is masked (val=0)
    outside the segment's slots, so the PE config never changes.

    Returns (quota, slot0, runs, segs, seg_map, first, last, T, S):
      runs     [(si, q, t0, R)]
      segs     per batch t: [(b, s)] in emission order
      seg_map  [T, nb] -> s (or -1)
      first/last  {b: s} accumulation start/stop segment
    """
    nb, nq = counts.shape[1], counts.shape[2]
    quota = counts.max(axis=0).astype(np.int64)  # [nb, nq]
    for b in range(nb):
        if quota[b].sum() == 0:
            quota[b, 0] = 1
    sbs = [(u, min(u + sblk, nb)) for u in range(0, nb, sblk)]
    slot0 = np.full((nb, nq), -1, dtype=np.int64)
    runs = []
    spans = []  # per batch t: list of blocks b touching it
    T = 0
    for si, (u0, u1) in enumerate(sbs):
        for q in range(nq):
            tot = int(quota[u0:u1, q].sum())
            if tot == 0:
                continue
            R = -(-tot // P)
            t0 = T
            off = t0 * P
            for b in range(u0, u1):
                if quota[b, q] == 0:
                    continue
                slot0[b, q] = off
                s, e = off, off + int(quota[b, q])
                for k in range(s // P, (e - 1) // P + 1):
                    while len(spans) <= k:
                        spans.append([])
                    spans[k].append(b)
                off = e
            runs.append((si, q, t0, R))
            T += R
            while len(spans) < T:
                spans.append([])
    segs = []
    seg_map = np.full((T, nb), -1, dtype=np.int64)
    S = 0
    first, last = {}, {}
    for t in range(T):
        row = []
        for b in spans[t]:
            seg_map[t, b] = S
            row.append((b, S))
            if b not in first:
                first[b] = S
            last[b] = S
            S += 1
        segs.append(row)
    return quota, slot0, runs, segs, seg_map, first, last, T, S


def _prep(A_rows, A_cols, A_vals, n_cores, npc, ch, sblk):
    nb = (npc + P - 1) // P
    nq = len(ch) - 1
    core = A_rows // npc
    rl = A_rows - core * npc
    blk = rl // P
    dloc = rl % P
    q = np.searchsorted(ch, A_cols, side="right") - 1
    cell = (core * nb + blk) * nq + q
    counts = np.bincount(cell, minlength=n_cores * nb * nq).reshape(
        n_cores, nb, nq
    )
    quota, slot0, runs, segs, seg_map, first, last, T, S = _schedule(
        counts, sblk
    )
    slot0_flat = slot0.reshape(-1)
    metas = []
    for c in range(n_cores):
        m = core == c
        dloc_c, cols_c, vals_c, blk_c = dloc[m], A_cols[m], A_vals[m], blk[m]
        key = blk_c * nq + q[m]
        order = np.argsort(key, kind="stable")
        dloc_c, cols_c, vals_c, key, blk_c = (
            dloc_c[order],
            cols_c[order],
            vals_c[order],
            key[order],
            blk_c[order],
        )
        ccounts = counts[c].reshape(-1)
        starts = np.concatenate([[0], np.cumsum(ccounts)])[:-1]
        pos = np.arange(key.size) - starts[key]
        slot = slot0_flat[key] + pos
        assert (slot >= 0).all()
        t_of = slot // P
        i_of = slot % P
        s_of = seg_map[t_of, blk_c]
        assert (s_of >= 0).all()
        idx16 = (cols_c - ch[q[m][order]]).astype(np.int16)
        idx_flat = np.zeros((16, 8 * T), np.int16)
        idx_flat[i_of % 16, t_of * 8 + i_of // 16] = idx16
        idx_all = np.tile(idx_flat, (8, 1))
        dloc_t = np.zeros((P, S), np.float32)
        val_t = np.zeros((P, S), np.float32)
        dloc_t[i_of, s_of] = dloc_c.astype(np.float32)
        val_t[i_of, s_of] = vals_c
        metas.append((idx_all, dloc_t, val_t))
    return metas, (quota, slot0, runs, segs, first, last, T, S), nb, nq


def _build_program(
    n_nodes, ch, sched, nb, sblk, reps=1,
    do_gather=True, do_oh=True, do_mm=True, do_proj=True,
):
    quota, slot0, runs, segs, first, last, T, S = sched
    if not do_mm:
        do_proj = False  # proj reads the PSUM accumulators mm writes
    sbs = [(u, min(u + sblk, nb)) for u in range(0, nb, sblk)]
    nc = bacc.Bacc(
        "TRN2", target_bir_lowering=False, debug=False, num_devices=N_CORES,
        num_swdge_queues=4, dynamic_dma_scratch_size=SCRATCH,
    )
    x_d = nc.dram_tensor("X", [n_nodes, D], XDT, kind="ExternalInput").ap()
    idx_d = nc.dram_tensor("idx", [P, 8 * T], I16, kind="ExternalInput").ap()
    dloc_d = nc.dram_tensor("dloc", [P, S], F32, kind="ExternalInput").ap()
    val_d = nc.dram_tensor("val", [P, S], F32, kind="ExternalInput").ap()
    wt_d = nc.dram_tensor("wt", [P, D], BF16, kind="ExternalInput").ap()
    bb_d = nc.dram_tensor("bb", [P, D], F32, kind="ExternalInput").ap()
    iota_d = nc.dram_tensor("iota", [P, P], BF16, kind="ExternalInput").ap()
    y_d = nc.dram_tensor("y", [nb * P, D], F32, kind="ExternalOutput").ap()

    qrr = 0  # SWDGE queue round-robin
    with tile.TileContext(nc) as tc:
        with (
            tc.tile_pool(name="const", bufs=1) as cpool,
            tc.tile_pool(name="xg", bufs=2) as xgpool,
            tc.tile_pool(name="oh", bufs=4) as ohpool,
            tc.tile_pool(name="hts", bufs=3) as htspool,
            tc.tile_pool(name="yst", bufs=2) as ystpool,
            tc.tile_pool(name="psh", bufs=7, space="PSUM") as phpool,
            tc.tile_pool(name="psy", bufs=1, space="PSUM") as pypool,
        ):
            idx_s = cpool.tile([P, 8 * T], I16)
            nc.sync.dma_start(out=idx_s[:], in_=idx_d[:])
            dloc_s = cpool.tile([P, S], F32)
            nc.sync.dma_start(out=dloc_s[:], in_=dloc_d[:])
            val_s = cpool.tile([P, S], F32)
            nc.sync.dma_start(out=val_s[:], in_=val_d[:])
            wt_s = cpool.tile([P, D], BF16)
            nc.sync.dma_start(out=wt_s[:], in_=wt_d[:])
            bb_s = cpool.tile([P, D], F32)
            nc.sync.dma_start(out=bb_s[:], in_=bb_d[:])
            iota_s = cpool.tile([P, P], BF16)
            nc.sync.dma_start(out=iota_s[:], in_=iota_d[:])

            for rep in range(reps):
                hp = {}  # (si, gi) -> PSUM tile holding 4 block columns
                ystage = None
                cur_si = -1
                for si, q, t0, R in runs:
                    u0, u1 = sbs[si]
                    if si != cur_si:
                        if cur_si >= 0 and do_proj:
                            pu0, pu1 = sbs[cur_si]
                            g = pu1 - pu0
                            rows = y_d[pu0 * P : pu1 * P, :]
                            nc.sync.dma_start(
                                out=rows.rearrange("(g p) f -> p g f", p=P),
                                in_=ystage[:, : g * D].rearrange(
                                    "p (g f) -> p g f", f=D
                                ),
                            )
                        cur_si = si
                        if do_proj:
                            ystage = ystpool.tile(
                                [P, (u1 - u0) * D], F32, tag="yst"
                            )
                    xg = None
                    if do_gather:
                        xg = xgpool.tile([P, R * D], XDT, tag="xg")
                        for j0 in range(0, R, GB):
                            j1 = min(R, j0 + GB)
                            nc.gpsimd.dma_gather(
                                out_ap=xg[:, j0 * D : j1 * D].rearrange(
                                    "p (g f) -> p g f", f=D
                                ),
                                in_ap=x_d[int(ch[q]) : int(ch[q + 1]), :],
                                idxs_ap=idx_s[:, (t0 + j0) * 8 : (t0 + j1) * 8],
                                num_idxs=(j1 - j0) * P,
                                num_idxs_reg=(j1 - j0) * P,
                                elem_size=D,
                                single_packet=False,
                                queue_num=qrr % 4,
                            )
                            qrr += 1
                    for k in range(R):
                        t = t0 + k
                        for b, s in segs[t]:
                            gi = b - u0
                            ci = 0
                            oh = None
                            if do_oh:
                                oh = ohpool.tile([P, D], BF16, tag="oh")
                                nc.vector.tensor_scalar(
                                    out=oh[:],
                                    in0=iota_s[:],
                                    scalar1=dloc_s[:, s : s + 1],
                                    scalar2=val_s[:, s : s + 1],
                                    op0=mybir.AluOpType.is_equal,
                                    op1=mybir.AluOpType.mult,
                                )
                            if first[b] == s and do_mm:
                                if (si, gi) not in hp:
                                    hp[(si, gi)] = phpool.tile(
                                        [P, D], F32, tag="hp",
                                        name=f"hp{rep}_{si}_{gi}",
                                    )
                            if do_mm:
                                nc.tensor.matmul(
                                    out=hp[(si, gi)][:, ci * D : (ci + 1) * D],
                                    lhsT=xg[:, k * D : (k + 1) * D]
                                    if do_gather else iota_s[:],
                                    rhs=oh[:] if do_oh else iota_s[:],
                                    start=(first[b] == s),
                                    stop=(last[b] == s),
                                )
                            if last[b] == s and do_proj:
                                hts = htspool.tile([P, D], BF16, tag="hts")
                                nc.scalar.activation(
                                    out=hts[:],
                                    in_=hp[(si, gi)][:, ci * D : (ci + 1) * D],
                                    func=mybir.ActivationFunctionType.Copy,
                                )
                                yps = pypool.tile([P, D], F32, tag="yp")
                                nc.tensor.matmul(
                                    out=yps[:],
                                    lhsT=hts[:],
                                    rhs=wt_s[:],
                                    start=True,
                                    stop=True,
                                )
                                g = b - u0
                                nc.vector.tensor_tensor(
                                    out=ystage[:, g * D : (g + 1) * D],
                                    in0=yps[:],
                                    in1=bb_s[:],
                                    op=mybir.AluOpType.add,
                                )
                if do_proj:
                    pu0, pu1 = sbs[cur_si]
                    g = pu1 - pu0
                    rows = y_d[pu0 * P : pu1 * P, :]
                    nc.sync.dma_start(
                        out=rows.rearrange("(g p) f -> p g f", p=P),
                        in_=ystage[:, : g * D].rearrange("p (g f) -> p g f", f=D),
                    )
    nc.finalize()
    return nc


def _make_in_maps(inputs, n_cores=N_CORES, npc=NPC, chunk=CHUNK, sblk=SBLK):
    X = np.asarray(inputs["X"], dtype=np.float32)
    A_rows = np.asarray(inputs["A_rows"], dtype=np.int64)
    A_cols = np.asarray(inputs["A_cols"], dtype=np.int64)
    A_vals = np.asarray(inputs["A_vals"], dtype=np.float32)
    W = np.asarray(inputs["W"], dtype=np.float32)
    bias = np.asarray(inputs["b"], dtype=np.float32)

    n_nodes = X.shape[0]
    ch = _chunk_bounds(n_nodes, chunk)
    metas, sched, nb, _ = _prep(A_rows, A_cols, A_vals, n_cores, npc, ch, sblk)
    xbf = np.ascontiguousarray(X.astype(mybir.dt.np(XDT)))
    wt = np.ascontiguousarray(W.T.astype(NP_BF16))
    bb = np.broadcast_to(bias[None, :], (P, D)).astype(np.float32).copy()
    iota = np.broadcast_to(
        np.arange(P, dtype=np.float32)[None, :], (P, P)
    ).astype(NP_BF16).copy()
    in_maps = []
    for idx_all, dloc_t, val_t in metas:
        in_maps.append(
            {
                "X": xbf,
                "idx": idx_all,
                "dloc": dloc_t,
                "val": val_t,
                "wt": wt,
                "bb": bb,
                "iota": iota,
            }
        )
    return in_maps, ch, sched, nb


def _run(inputs, trace=False, **kw):
    in_maps, ch, sched, nb = _make_in_maps(inputs)
    nc = _build_program(np.asarray(inputs["X"]).shape[0], ch, sched, nb, SBLK)
    res = run_bass_kernel_spmd(nc, in_maps, list(range(N_CORES)), trace=trace, **kw)
    out = np.concatenate([res.results[c]["y"][:NPC] for c in range(N_CORES)], axis=0)
    return out, res


def kernel(**inputs):
    return _run(inputs, trace=False)[0]


# revision 20
# speedup vs baseline: 1.5351x; 1.5351x over previous
"""GCN layer (COO SpMM + linear) on 8 Trainium2 NeuronCores.

Strategy (per sharding hint): shard destination nodes across the 8 cores
(12,500 rows each); partition edges by destination so the segment-sum is
core-local; replicate X (each core gathers source rows from its own full
copy in HBM, stored bf16) and the small [128,128] weight.

Per-core kernel:
  - dest nodes are grouped into blocks of 128; blocks into super-blocks of
    SBLK=28 (4 block accumulators packed per PSUM bank -> 7 banks).
  - dma_gather uses int16 indices, so X is windowed into chunks of 32768
    rows. Edge slots are laid out per (super-block, chunk) run, block by
    block, with each (block, chunk) cell's slot quota rounded to 16 (max
    over the 8 cores, so one SPMD program serves all of them). Batches of
    128 slots may span block boundaries; the per-batch matmul is split
    into 16-aligned partition sub-ranges, one per touched block.
  - per batch (128 edge slots, one per SBUF partition):
      Xg   = dma_gather of bf16 X[col[e]] rows        [128e, 128f]
      S_T  = val[e] * (iota[d] == dloc[e])   (one fused DVE tensor_scalar,
             all-bf16 operands -> 4x DVE mode)
      h.T += Xg.T @ S_T   (bf16 PE matmul into the block's PSUM column)
  - per block: y = (h.T).T @ W.T + b via a second matmul, staged per
    super-block and DMA'd out in fp32.
"""

import sys

import numpy as np

sys.path.insert(0, "/opt/trn_rl_repo")

import concourse.bacc as bacc
import concourse.mybir as mybir
import concourse.tile as tile
from concourse.bass_utils import run_bass_kernel_spmd

N_NODES = 100000
D = 128
N_CORES = 8
NPC = N_NODES // N_CORES  # nodes per core
P = 128
CHUNK = 32768  # int16 index window over X rows
SBLK = 7  # blocks per super-block (one PSUM bank per block + 1 proj bank)
GB = 64  # batches per dma_gather instruction (ring entries ~ num_idxs/16)
QUANT = 32  # slot quantum per (block, chunk) cell (PE tile_position rule)
SCRATCH = 16384  # SWDGE ring: SCRATCH/16 = 1024 entries/queue (gather uses idxs/16+1)

F32 = mybir.dt.float32
BF16 = mybir.dt.bfloat16
I16 = mybir.dt.int16
NP_BF16 = mybir.dt.np(BF16)
XDT = BF16  # gather payload dtype (BF16 or F32; F32 only for experiments)
SINGLE_PACKET = False  # dma_gather single_packet knob


def _chunk_bounds(n_nodes, chunk):
    ch = list(range(0, n_nodes, chunk)) + [n_nodes]
    return np.array(ch, dtype=np.int64)


def _mm_pieces(lo, hi):
    """Split [lo, hi) into PE-legal (tile_position) partition ranges:
    size<=32 may start at 0/32/64/96, size<=64 at 0/64, larger at 0."""
    pieces = []
    while lo < hi:
        if lo == 0 or lo == 64 and hi - lo <= 64:
            pieces.append((lo, hi))
            break
        nxt = min(hi, lo + 32)
        pieces.append((lo, nxt))
        lo = nxt
    return pieces


def _schedule(counts, sblk):
    """counts: [n_cores, nb, nq] -> shared slot/segment schedule.

    Slots for cell (b, q) = max edge count over the 8 cores (exact, no
    rounding). Batches of 128 slots span blocks freely; each (batch, block)
    intersection is a SEGMENT with its own global index s. Every segment
    runs as a full-128-partition matmul whose one-hot is masked (val=0)
    outside the segment's slots, so the PE config never changes.

    Returns (quota, slot0, runs, segs, seg_map, first, last, T, S):
      runs     [(si, q, t0, R)]
      segs     per batch t: [(b, s)] in emission order
      seg_map  [T, nb] -> s (or -1)
      first/last  {b: s} accumulation start/stop segment
    """
    nb, nq = counts.shape[1], counts.shape[2]
    quota = counts.max(axis=0).astype(np.int64)  # [nb, nq]
    for b in range(nb):
        if quota[b].sum() == 0:
            quota[b, 0] = 1
    sbs = [(u, min(u + sblk, nb)) for u in range(0, nb, sblk)]
    slot0 = np.full((nb, nq), -1, dtype=np.int64)
    runs = []
    spans = []  # per batch t: list of blocks b touching it
    T = 0
    for si, (u0, u1) in enumerate(sbs):
        for q in range(nq):
            tot = int(quota[u0:u1, q].sum())
            if tot == 0:
                continue
            R = -(-tot // P)
            t0 = T
            off = t0 * P
            for b in range(u0, u1):
                if quota[b, q] == 0:
                    continue
                slot0[b, q] = off
                s, e = off, off + int(quota[b, q])
                for k in range(s // P, (e - 1) // P + 1):
                    while len(spans) <= k:
                        spans.append([])
                    spans[k].append(b)
                off = e
            runs.append((si, q, t0, R))
            T += R
            while len(spans) < T:
                spans.append([])
    segs = []
    seg_map = np.full((T, nb), -1, dtype=np.int64)
    S = 0
    first, last = {}, {}
    for t in range(T):
        row = []
        for b in spans[t]:
            seg_map[t, b] = S
            row.append((b, S))
            if b not in first:
                first[b] = S
            last[b] = S
            S += 1
        segs.append(row)
    return quota, slot0, runs, segs, seg_map, first, last, T, S


def _prep(A_rows, A_cols, A_vals, n_cores, npc, ch, sblk):
    nb = (npc + P - 1) // P
    nq = len(ch) - 1
    core = A_rows // npc
    rl = A_rows - core * npc
    blk = rl // P
    dloc = rl % P
    q = np.searchsorted(ch, A_cols, side="right") - 1
    cell = (core * nb + blk) * nq + q
    counts = np.bincount(cell, minlength=n_cores * nb * nq).reshape(
        n_cores, nb, nq
    )
    quota, slot0, runs, segs, seg_map, first, last, T, S = _schedule(
        counts, sblk
    )
    slot0_flat = slot0.reshape(-1)
    metas = []
    for c in range(n_cores):
        m = core == c
        dloc_c, cols_c, vals_c, blk_c = dloc[m], A_cols[m], A_vals[m], blk[m]
        key = blk_c * nq + q[m]
        order = np.argsort(key, kind="stable")
        dloc_c, cols_c, vals_c, key, blk_c = (
            dloc_c[order],
            cols_c[order],
            vals_c[order],
            key[order],
            blk_c[order],
        )
        ccounts = counts[c].reshape(-1)
        starts = np.concatenate([[0], np.cumsum(ccounts)])[:-1]
        pos = np.arange(key.size) - starts[key]
        slot = slot0_flat[key] + pos
        assert (slot >= 0).all()
        t_of = slot // P
        i_of = slot % P
        s_of = seg_map[t_of, blk_c]
        assert (s_of >= 0).all()
        idx16 = (cols_c - ch[q[m][order]]).astype(np.int16)
        idx_flat = np.zeros((16, 8 * T), np.int16)
        idx_flat[i_of % 16, t_of * 8 + i_of // 16] = idx16
        idx_all = np.tile(idx_flat, (8, 1))
        dloc_t = np.zeros((P, S), np.float32)
        val_t = np.zeros((P, S), np.float32)
        dloc_t[i_of, s_of] = dloc_c.astype(np.float32)
        val_t[i_of, s_of] = vals_c
        metas.append((idx_all, dloc_t, val_t))
    return metas, (quota, slot0, runs, segs, first, last, T, S), nb, nq


def _build_program(
    n_nodes, ch, sched, nb, sblk, reps=1,
    do_gather=True, do_oh=True, do_mm=True, do_proj=True,
):
    quota, slot0, runs, segs, first, last, T, S = sched
    if not do_mm:
        do_proj = False  # proj reads the PSUM accumulators mm writes
    sbs = [(u, min(u + sblk, nb)) for u in range(0, nb, sblk)]
    nc = bacc.Bacc(
        "TRN2", target_bir_lowering=False, debug=False, num_devices=N_CORES,
        num_swdge_queues=4, dynamic_dma_scratch_size=SCRATCH,
    )
    x_d = nc.dram_tensor("X", [n_nodes, D], XDT, kind="ExternalInput").ap()
    idx_d = nc.dram_tensor("idx", [P, 8 * T], I16, kind="ExternalInput").ap()
    dloc_d = nc.dram_tensor("dloc", [P, S], F32, kind="ExternalInput").ap()
    val_d = nc.dram_tensor("val", [P, S], F32, kind="ExternalInput").ap()
    wt_d = nc.dram_tensor("wt", [P, D], BF16, kind="ExternalInput").ap()
    bb_d = nc.dram_tensor("bb", [P, D], F32, kind="ExternalInput").ap()
    iota_d = nc.dram_tensor("iota", [P, P], BF16, kind="ExternalInput").ap()
    y_d = nc.dram_tensor("y", [nb * P, D], F32, kind="ExternalOutput").ap()

    qrr = 0  # SWDGE queue round-robin
    with tile.TileContext(nc) as tc:
        with (
            tc.tile_pool(name="const", bufs=1) as cpool,
            tc.tile_pool(name="xg", bufs=2) as xgpool,
            tc.tile_pool(name="oh", bufs=4) as ohpool,
            tc.tile_pool(name="hts", bufs=3) as htspool,
            tc.tile_pool(name="yst", bufs=2) as ystpool,
            tc.tile_pool(name="psh", bufs=7, space="PSUM") as phpool,
            tc.tile_pool(name="psy", bufs=1, space="PSUM") as pypool,
        ):
            idx_s = cpool.tile([P, 8 * T], I16)
            nc.sync.dma_start(out=idx_s[:], in_=idx_d[:])
            dloc_s = cpool.tile([P, S], F32)
            nc.sync.dma_start(out=dloc_s[:], in_=dloc_d[:])
            val_s = cpool.tile([P, S], F32)
            nc.sync.dma_start(out=val_s[:], in_=val_d[:])
            wt_s = cpool.tile([P, D], BF16)
            nc.sync.dma_start(out=wt_s[:], in_=wt_d[:])
            bb_s = cpool.tile([P, D], F32)
            nc.sync.dma_start(out=bb_s[:], in_=bb_d[:])
            iota_s = cpool.tile([P, P], BF16)
            nc.sync.dma_start(out=iota_s[:], in_=iota_d[:])

            for rep in range(reps):
                hp = {}  # (si, gi) -> PSUM tile holding 4 block columns
                ystage = None
                cur_si = -1
                for si, q, t0, R in runs:
                    u0, u1 = sbs[si]
                    if si != cur_si:
                        if cur_si >= 0 and do_proj:
                            pu0, pu1 = sbs[cur_si]
                            g = pu1 - pu0
                            rows = y_d[pu0 * P : pu1 * P, :]
                            nc.sync.dma_start(
                                out=rows.rearrange("(g p) f -> p g f", p=P),
                                in_=ystage[:, : g * D].rearrange(
                                    "p (g f) -> p g f", f=D
                                ),
                            )
                        cur_si = si
                        if do_proj:
                            ystage = ystpool.tile(
                                [P, (u1 - u0) * D], F32, tag="yst"
                            )
                    xg = None
                    if do_gather:
                        xg = xgpool.tile([P, R * D], XDT, tag="xg")
                        nparts = min(4, R)
                        part = -(-R // nparts)
                        for pi in range(nparts):
                            j0 = pi * part
                            j1 = min(R, j0 + part)
                            if j0 >= j1:
                                continue
                            nc.gpsimd.dma_gather(
                                out_ap=xg[:, j0 * D : j1 * D].rearrange(
                                    "p (g f) -> p g f", f=D
                                ),
                                in_ap=x_d[int(ch[q]) : int(ch[q + 1]), :],
                                idxs_ap=idx_s[:, (t0 + j0) * 8 : (t0 + j1) * 8],
                                num_idxs=(j1 - j0) * P,
                                num_idxs_reg=(j1 - j0) * P,
                                elem_size=D,
                                single_packet=SINGLE_PACKET,
                                queue_num=pi,
                            )
                    for k in range(R):
                        t = t0 + k
                        for b, s in segs[t]:
                            gi = b - u0
                            ci = 0
                            oh = None
                            if do_oh:
                                oh = ohpool.tile([P, D], BF16, tag="oh")
                                nc.vector.tensor_scalar(
                                    out=oh[:],
                                    in0=iota_s[:],
                                    scalar1=dloc_s[:, s : s + 1],
                                    scalar2=val_s[:, s : s + 1],
                                    op0=mybir.AluOpType.is_equal,
                                    op1=mybir.AluOpType.mult,
                                )
                            if first[b] == s and do_mm:
                                if (si, gi) not in hp:
                                    hp[(si, gi)] = phpool.tile(
                                        [P, D], F32, tag="hp",
                                        name=f"hp{rep}_{si}_{gi}",
                                    )
                            if do_mm:
                                nc.tensor.matmul(
                                    out=hp[(si, gi)][:, ci * D : (ci + 1) * D],
                                    lhsT=xg[:, k * D : (k + 1) * D]
                                    if do_gather else iota_s[:],
                                    rhs=oh[:] if do_oh else iota_s[:],
                                    start=(first[b] == s),
                                    stop=(last[b] == s),
                                )
                            if last[b] == s and do_proj:
                                hts = htspool.tile([P, D], BF16, tag="hts")
                                nc.scalar.activation(
                                    out=hts[:],
                                    in_=hp[(si, gi)][:, ci * D : (ci + 1) * D],
                                    func=mybir.ActivationFunctionType.Copy,
                                )
                                yps = pypool.tile([P, D], F32, tag="yp")
                                nc.tensor.matmul(
                                    out=yps[:],
                                    lhsT=hts[:],
                                    rhs=wt_s[:],
                                    start=True,
                                    stop=True,
                                )
                                g = b - u0
                                nc.vector.tensor_tensor(
                                    out=ystage[:, g * D : (g + 1) * D],
                                    in0=yps[:],
                                    in1=bb_s[:],
                                    op=mybir.AluOpType.add,
                                )
                if do_proj:
                    pu0, pu1 = sbs[cur_si]
                    g = pu1 - pu0
                    rows = y_d[pu0 * P : pu1 * P, :]
                    nc.sync.dma_start(
                        out=rows.rearrange("(g p) f -> p g f", p=P),
                        in_=ystage[:, : g * D].rearrange("p (g f) -> p g f", f=D),
                    )
    nc.finalize()
    return nc


def _make_in_maps(inputs, n_cores=N_CORES, npc=NPC, chunk=CHUNK, sblk=SBLK):
    X = np.asarray(inputs["X"], dtype=np.float32)
    A_rows = np.asarray(inputs["A_rows"], dtype=np.int64)
    A_cols = np.asarray(inputs["A_cols"], dtype=np.int64)
    A_vals = np.asarray(inputs["A_vals"], dtype=np.float32)
    W = np.asarray(inputs["W"], dtype=np.float32)
    bias = np.asarray(inputs["b"], dtype=np.float32)

    n_nodes = X.shape[0]
    ch = _chunk_bounds(n_nodes, chunk)
    metas, sched, nb, _ = _prep(A_rows, A_cols, A_vals, n_cores, npc, ch, sblk)
    xbf = np.ascontiguousarray(X.astype(mybir.dt.np(XDT)))
    wt = np.ascontiguousarray(W.T.astype(NP_BF16))
    bb = np.broadcast_to(bias[None, :], (P, D)).astype(np.float32).copy()
    iota = np.broadcast_to(
        np.arange(P, dtype=np.float32)[None, :], (P, P)
    ).astype(NP_BF16).copy()
    in_maps = []
    for idx_all, dloc_t, val_t in metas:
        in_maps.append(
            {
                "X": xbf,
                "idx": idx_all,
                "dloc": dloc_t,
                "val": val_t,
                "wt": wt,
                "bb": bb,
                "iota": iota,
            }
        )
    return in_maps, ch, sched, nb


def _run(inputs, trace=False, **kw):
    in_maps, ch, sched, nb = _make_in_maps(inputs)
    nc = _build_program(np.asarray(inputs["X"]).shape[0], ch, sched, nb, SBLK)
    res = run_bass_kernel_spmd(nc, in_maps, list(range(N_CORES)), trace=trace, **kw)
    out = np.concatenate([res.results[c]["y"][:NPC] for c in range(N_CORES)], axis=0)
    return out, res


def kernel(**inputs):
    return _run(inputs, trace=False)[0]


# revision 21
# speedup vs baseline: 1.6221x; 1.0567x over previous
"""GCN layer (COO SpMM + linear) on 8 Trainium2 NeuronCores.

Strategy (per sharding hint): shard destination nodes across the 8 cores
(12,500 rows each); partition edges by destination so the segment-sum is
core-local; replicate X (each core gathers source rows from its own full
copy in HBM, stored bf16) and the small [128,128] weight.

Per-core kernel:
  - dest nodes are grouped into blocks of 128; blocks into super-blocks of
    SBLK=28 (4 block accumulators packed per PSUM bank -> 7 banks).
  - dma_gather uses int16 indices, so X is windowed into chunks of 32768
    rows. Edge slots are laid out per (super-block, chunk) run, block by
    block, with each (block, chunk) cell's slot quota rounded to 16 (max
    over the 8 cores, so one SPMD program serves all of them). Batches of
    128 slots may span block boundaries; the per-batch matmul is split
    into 16-aligned partition sub-ranges, one per touched block.
  - per batch (128 edge slots, one per SBUF partition):
      Xg   = dma_gather of bf16 X[col[e]] rows        [128e, 128f]
      S_T  = val[e] * (iota[d] == dloc[e])   (one fused DVE tensor_scalar,
             all-bf16 operands -> 4x DVE mode)
      h.T += Xg.T @ S_T   (bf16 PE matmul into the block's PSUM column)
  - per block: y = (h.T).T @ W.T + b via a second matmul, staged per
    super-block and DMA'd out in fp32.
"""

import sys

import numpy as np

sys.path.insert(0, "/opt/trn_rl_repo")

import concourse.bacc as bacc
import concourse.mybir as mybir
import concourse.tile as tile
from concourse.bass_utils import run_bass_kernel_spmd

N_NODES = 100000
D = 128
N_CORES = 8
NPC = N_NODES // N_CORES  # nodes per core
P = 128
CHUNK = 32768  # int16 index window over X rows
SBLK = 7  # blocks per super-block (one PSUM bank per block + 1 proj bank)
GB = 64  # batches per dma_gather instruction (ring entries ~ num_idxs/16)
QUANT = 32  # slot quantum per (block, chunk) cell (PE tile_position rule)
SCRATCH = 16384  # SWDGE ring: SCRATCH/16 = 1024 entries/queue (gather uses idxs/16+1)

F32 = mybir.dt.float32
BF16 = mybir.dt.bfloat16
I16 = mybir.dt.int16
NP_BF16 = mybir.dt.np(BF16)
XDT = BF16  # gather payload dtype (BF16 or F32; F32 only for experiments)
SINGLE_PACKET = False  # dma_gather single_packet knob


def _chunk_bounds(n_nodes, chunk):
    ch = list(range(0, n_nodes, chunk)) + [n_nodes]
    return np.array(ch, dtype=np.int64)


def _mm_pieces(lo, hi):
    """Split [lo, hi) into PE-legal (tile_position) partition ranges:
    size<=32 may start at 0/32/64/96, size<=64 at 0/64, larger at 0."""
    pieces = []
    while lo < hi:
        if lo == 0 or lo == 64 and hi - lo <= 64:
            pieces.append((lo, hi))
            break
        nxt = min(hi, lo + 32)
        pieces.append((lo, nxt))
        lo = nxt
    return pieces


def _schedule(counts, sblk):
    """counts: [n_cores, nb, nq] -> shared slot/segment schedule.

    Slots for cell (b, q) = max edge count over the 8 cores (exact, no
    rounding). Batches of 128 slots span blocks freely; each (batch, block)
    intersection is a SEGMENT with its own global index s. Every segment
    runs as a full-128-partition matmul whose one-hot is masked (val=0)
    outside the segment's slots, so the PE config never changes.

    Returns (quota, slot0, runs, segs, seg_map, first, last, T, S):
      runs     [(si, q, t0, R)]
      segs     per batch t: [(b, s)] in emission order
      seg_map  [T, nb] -> s (or -1)
      first/last  {b: s} accumulation start/stop segment
    """
    nb, nq = counts.shape[1], counts.shape[2]
    quota = counts.max(axis=0).astype(np.int64)  # [nb, nq]
    for b in range(nb):
        if quota[b].sum() == 0:
            quota[b, 0] = 1
    sbs = [(u, min(u + sblk, nb)) for u in range(0, nb, sblk)]
    slot0 = np.full((nb, nq), -1, dtype=np.int64)
    runs = []
    spans = []  # per batch t: list of blocks b touching it
    T = 0
    for si, (u0, u1) in enumerate(sbs):
        for q in range(nq):
            tot = int(quota[u0:u1, q].sum())
            if tot == 0:
                continue
            R = -(-tot // P)
            t0 = T
            off = t0 * P
            for b in range(u0, u1):
                if quota[b, q] == 0:
                    continue
                slot0[b, q] = off
                s, e = off, off + int(quota[b, q])
                for k in range(s // P, (e - 1) // P + 1):
                    while len(spans) <= k:
                        spans.append([])
                    spans[k].append(b)
                off = e
            runs.append((si, q, t0, R))
            T += R
            while len(spans) < T:
                spans.append([])
    segs = []
    seg_map = np.full((T, nb), -1, dtype=np.int64)
    S = 0
    first, last = {}, {}
    for t in range(T):
        row = []
        for b in spans[t]:
            seg_map[t, b] = S
            row.append((b, S))
            if b not in first:
                first[b] = S
            last[b] = S
            S += 1
        segs.append(row)
    return quota, slot0, runs, segs, seg_map, first, last, T, S


def _prep(A_rows, A_cols, A_vals, n_cores, npc, ch, sblk):
    nb = (npc + P - 1) // P
    nq = len(ch) - 1
    core = A_rows // npc
    rl = A_rows - core * npc
    blk = rl // P
    dloc = rl % P
    q = np.searchsorted(ch, A_cols, side="right") - 1
    cell = (core * nb + blk) * nq + q
    counts = np.bincount(cell, minlength=n_cores * nb * nq).reshape(
        n_cores, nb, nq
    )
    quota, slot0, runs, segs, seg_map, first, last, T, S = _schedule(
        counts, sblk
    )
    slot0_flat = slot0.reshape(-1)
    metas = []
    for c in range(n_cores):
        m = core == c
        dloc_c, cols_c, vals_c, blk_c = dloc[m], A_cols[m], A_vals[m], blk[m]
        key = blk_c * nq + q[m]
        order = np.argsort(key, kind="stable")
        dloc_c, cols_c, vals_c, key, blk_c = (
            dloc_c[order],
            cols_c[order],
            vals_c[order],
            key[order],
            blk_c[order],
        )
        ccounts = counts[c].reshape(-1)
        starts = np.concatenate([[0], np.cumsum(ccounts)])[:-1]
        pos = np.arange(key.size) - starts[key]
        slot = slot0_flat[key] + pos
        assert (slot >= 0).all()
        t_of = slot // P
        i_of = slot % P
        s_of = seg_map[t_of, blk_c]
        assert (s_of >= 0).all()
        idx16 = (cols_c - ch[q[m][order]]).astype(np.int16)
        idx_flat = np.zeros((16, 8 * T), np.int16)
        idx_flat[i_of % 16, t_of * 8 + i_of // 16] = idx16
        idx_all = np.tile(idx_flat, (8, 1))
        dloc_t = np.zeros((P, S), np.float32)
        val_t = np.zeros((P, S), np.float32)
        dloc_t[i_of, s_of] = dloc_c.astype(np.float32)
        val_t[i_of, s_of] = vals_c
        metas.append((idx_all, dloc_t, val_t))
    return metas, (quota, slot0, runs, segs, first, last, T, S), nb, nq


def _build_program(
    n_nodes, ch, sched, nb, sblk, reps=1,
    do_gather=True, do_oh=True, do_mm=True, do_proj=True,
):
    quota, slot0, runs, segs, first, last, T, S = sched
    if not do_mm:
        do_proj = False  # proj reads the PSUM accumulators mm writes
    sbs = [(u, min(u + sblk, nb)) for u in range(0, nb, sblk)]
    nc = bacc.Bacc(
        "TRN2", target_bir_lowering=False, debug=False, num_devices=N_CORES,
        num_swdge_queues=4, dynamic_dma_scratch_size=SCRATCH,
    )
    x_d = nc.dram_tensor("X", [n_nodes, D], XDT, kind="ExternalInput").ap()
    idx_d = nc.dram_tensor("idx", [P, 8 * T], I16, kind="ExternalInput").ap()
    dloc_d = nc.dram_tensor("dloc", [P, S], F32, kind="ExternalInput").ap()
    val_d = nc.dram_tensor("val", [P, S], F32, kind="ExternalInput").ap()
    wt_d = nc.dram_tensor("wt", [P, D], BF16, kind="ExternalInput").ap()
    bb_d = nc.dram_tensor("bb", [P, D], F32, kind="ExternalInput").ap()
    iota_d = nc.dram_tensor("iota", [P, P], BF16, kind="ExternalInput").ap()
    y_d = nc.dram_tensor("y", [nb * P, D], F32, kind="ExternalOutput").ap()

    qrr = 0  # SWDGE queue round-robin
    with tile.TileContext(nc) as tc:
        with (
            tc.tile_pool(name="const", bufs=1) as cpool,
            tc.tile_pool(name="xg", bufs=3) as xgpool,
            tc.tile_pool(name="oh", bufs=8) as ohpool,
            tc.tile_pool(name="hts", bufs=4) as htspool,
            tc.tile_pool(name="yst", bufs=2) as ystpool,
            tc.tile_pool(name="psh", bufs=7, space="PSUM") as phpool,
            tc.tile_pool(name="psy", bufs=1, space="PSUM") as pypool,
        ):
            idx_s = cpool.tile([P, 8 * T], I16)
            nc.sync.dma_start(out=idx_s[:], in_=idx_d[:])
            dloc_s = cpool.tile([P, S], F32)
            nc.sync.dma_start(out=dloc_s[:], in_=dloc_d[:])
            val_s = cpool.tile([P, S], F32)
            nc.sync.dma_start(out=val_s[:], in_=val_d[:])
            wt_s = cpool.tile([P, D], BF16)
            nc.sync.dma_start(out=wt_s[:], in_=wt_d[:])
            bb_s = cpool.tile([P, D], F32)
            nc.sync.dma_start(out=bb_s[:], in_=bb_d[:])
            iota_s = cpool.tile([P, P], BF16)
            nc.sync.dma_start(out=iota_s[:], in_=iota_d[:])

            for rep in range(reps):
                hp = {}  # (si, gi) -> PSUM tile holding 4 block columns
                ystage = None
                cur_si = -1
                for si, q, t0, R in runs:
                    u0, u1 = sbs[si]
                    if si != cur_si:
                        if cur_si >= 0 and do_proj:
                            pu0, pu1 = sbs[cur_si]
                            g = pu1 - pu0
                            rows = y_d[pu0 * P : pu1 * P, :]
                            nc.sync.dma_start(
                                out=rows.rearrange("(g p) f -> p g f", p=P),
                                in_=ystage[:, : g * D].rearrange(
                                    "p (g f) -> p g f", f=D
                                ),
                            )
                        cur_si = si
                        if do_proj:
                            ystage = ystpool.tile(
                                [P, (u1 - u0) * D], F32, tag="yst"
                            )
                    xg = None
                    if do_gather:
                        xg = xgpool.tile([P, R * D], XDT, tag="xg")
                        nparts = min(4, R)
                        part = -(-R // nparts)
                        for pi in range(nparts):
                            j0 = pi * part
                            j1 = min(R, j0 + part)
                            if j0 >= j1:
                                continue
                            nc.gpsimd.dma_gather(
                                out_ap=xg[:, j0 * D : j1 * D].rearrange(
                                    "p (g f) -> p g f", f=D
                                ),
                                in_ap=x_d[int(ch[q]) : int(ch[q + 1]), :],
                                idxs_ap=idx_s[:, (t0 + j0) * 8 : (t0 + j1) * 8],
                                num_idxs=(j1 - j0) * P,
                                num_idxs_reg=(j1 - j0) * P,
                                elem_size=D,
                                single_packet=SINGLE_PACKET,
                                queue_num=pi,
                            )
                    for k in range(R):
                        t = t0 + k
                        for b, s in segs[t]:
                            gi = b - u0
                            ci = 0
                            oh = None
                            if do_oh:
                                oh = ohpool.tile([P, D], BF16, tag="oh")
                                nc.vector.tensor_scalar(
                                    out=oh[:],
                                    in0=iota_s[:],
                                    scalar1=dloc_s[:, s : s + 1],
                                    scalar2=val_s[:, s : s + 1],
                                    op0=mybir.AluOpType.is_equal,
                                    op1=mybir.AluOpType.mult,
                                )
                            if first[b] == s and do_mm:
                                if (si, gi) not in hp:
                                    hp[(si, gi)] = phpool.tile(
                                        [P, D], F32, tag="hp",
                                        name=f"hp{rep}_{si}_{gi}",
                                    )
                            if do_mm:
                                nc.tensor.matmul(
                                    out=hp[(si, gi)][:, ci * D : (ci + 1) * D],
                                    lhsT=xg[:, k * D : (k + 1) * D]
                                    if do_gather else iota_s[:],
                                    rhs=oh[:] if do_oh else iota_s[:],
                                    start=(first[b] == s),
                                    stop=(last[b] == s),
                                )
                            if last[b] == s and do_proj:
                                hts = htspool.tile([P, D], BF16, tag="hts")
                                nc.scalar.activation(
                                    out=hts[:],
                                    in_=hp[(si, gi)][:, ci * D : (ci + 1) * D],
                                    func=mybir.ActivationFunctionType.Copy,
                                )
                                yps = pypool.tile([P, D], F32, tag="yp")
                                nc.tensor.matmul(
                                    out=yps[:],
                                    lhsT=hts[:],
                                    rhs=wt_s[:],
                                    start=True,
                                    stop=True,
                                )
                                g = b - u0
                                nc.vector.tensor_tensor(
                                    out=ystage[:, g * D : (g + 1) * D],
                                    in0=yps[:],
                                    in1=bb_s[:],
                                    op=mybir.AluOpType.add,
                                )
                if do_proj:
                    pu0, pu1 = sbs[cur_si]
                    g = pu1 - pu0
                    rows = y_d[pu0 * P : pu1 * P, :]
                    nc.sync.dma_start(
                        out=rows.rearrange("(g p) f -> p g f", p=P),
                        in_=ystage[:, : g * D].rearrange("p (g f) -> p g f", f=D),
                    )
    nc.finalize()
    return nc


def _make_in_maps(inputs, n_cores=N_CORES, npc=NPC, chunk=CHUNK, sblk=SBLK):
    X = np.asarray(inputs["X"], dtype=np.float32)
    A_rows = np.asarray(inputs["A_rows"], dtype=np.int64)
    A_cols = np.asarray(inputs["A_cols"], dtype=np.int64)
    A_vals = np.asarray(inputs["A_vals"], dtype=np.float32)
    W = np.asarray(inputs["W"], dtype=np.float32)
    bias = np.asarray(inputs["b"], dtype=np.float32)

    n_nodes = X.shape[0]
    ch = _chunk_bounds(n_nodes, chunk)
    metas, sched, nb, _ = _prep(A_rows, A_cols, A_vals, n_cores, npc, ch, sblk)
    xbf = np.ascontiguousarray(X.astype(mybir.dt.np(XDT)))
    wt = np.ascontiguousarray(W.T.astype(NP_BF16))
    bb = np.broadcast_to(bias[None, :], (P, D)).astype(np.float32).copy()
    iota = np.broadcast_to(
        np.arange(P, dtype=np.float32)[None, :], (P, P)
    ).astype(NP_BF16).copy()
    in_maps = []
    for idx_all, dloc_t, val_t in metas:
        in_maps.append(
            {
                "X": xbf,
                "idx": idx_all,
                "dloc": dloc_t,
                "val": val_t,
                "wt": wt,
                "bb": bb,
                "iota": iota,
            }
        )
    return in_maps, ch, sched, nb


def _run(inputs, trace=False, **kw):
    in_maps, ch, sched, nb = _make_in_maps(inputs)
    nc = _build_program(np.asarray(inputs["X"]).shape[0], ch, sched, nb, SBLK)
    res = run_bass_kernel_spmd(nc, in_maps, list(range(N_CORES)), trace=trace, **kw)
    out = np.concatenate([res.results[c]["y"][:NPC] for c in range(N_CORES)], axis=0)
    return out, res


def kernel(**inputs):
    return _run(inputs, trace=False)[0]


# revision 22
# speedup vs baseline: 2.6207x; 1.6156x over previous
"""GCN layer (COO SpMM + linear) on 8 Trainium2 NeuronCores.

Strategy (per sharding hint): shard destination nodes across the 8 cores
(12,500 rows each); partition edges by destination so the segment-sum is
core-local; replicate X (each core gathers source rows from its own full
copy in HBM, stored bf16) and the small [128,128] weight.

Per-core kernel:
  - dest nodes are grouped into blocks of 128; blocks into super-blocks of
    SBLK=28 (4 block accumulators packed per PSUM bank -> 7 banks).
  - dma_gather uses int16 indices, so X is windowed into chunks of 32768
    rows. Edge slots are laid out per (super-block, chunk) run, block by
    block, with each (block, chunk) cell's slot quota rounded to 16 (max
    over the 8 cores, so one SPMD program serves all of them). Batches of
    128 slots may span block boundaries; the per-batch matmul is split
    into 16-aligned partition sub-ranges, one per touched block.
  - per batch (128 edge slots, one per SBUF partition):
      Xg   = dma_gather of bf16 X[col[e]] rows        [128e, 128f]
      S_T  = val[e] * (iota[d] == dloc[e])   (one fused DVE tensor_scalar,
             all-bf16 operands -> 4x DVE mode)
      h.T += Xg.T @ S_T   (bf16 PE matmul into the block's PSUM column)
  - per block: y = (h.T).T @ W.T + b via a second matmul, staged per
    super-block and DMA'd out in fp32.
"""

import sys

import numpy as np

sys.path.insert(0, "/opt/trn_rl_repo")

import concourse.bacc as bacc
import concourse.mybir as mybir
import concourse.tile as tile
from concourse.bass_utils import run_bass_kernel_spmd

N_NODES = 100000
D = 128
N_CORES = 8
NPC = N_NODES // N_CORES  # nodes per core
P = 128
CHUNK = 32768  # int16 index window over X rows
SBLK = 6  # blocks per super-block (one PSUM bank per block + 2 proj banks)
GB = 64  # batches per dma_gather instruction (ring entries ~ num_idxs/16)
QUANT = 32  # slot quantum per (block, chunk) cell (PE tile_position rule)
SCRATCH = 16384  # SWDGE ring: SCRATCH/16 = 1024 entries/queue (gather uses idxs/16+1)

F32 = mybir.dt.float32
BF16 = mybir.dt.bfloat16
I16 = mybir.dt.int16
NP_BF16 = mybir.dt.np(BF16)
XDT = BF16  # gather payload dtype (BF16 or F32; F32 only for experiments)
SINGLE_PACKET = False  # dma_gather single_packet knob


def _chunk_bounds(n_nodes, chunk):
    ch = list(range(0, n_nodes, chunk)) + [n_nodes]
    return np.array(ch, dtype=np.int64)


def _mm_pieces(lo, hi):
    """Split [lo, hi) into PE-legal (tile_position) partition ranges:
    size<=32 may start at 0/32/64/96, size<=64 at 0/64, larger at 0."""
    pieces = []
    while lo < hi:
        if lo == 0 or lo == 64 and hi - lo <= 64:
            pieces.append((lo, hi))
            break
        nxt = min(hi, lo + 32)
        pieces.append((lo, nxt))
        lo = nxt
    return pieces


def _schedule(counts, sblk):
    """counts: [n_cores, nb, nq] -> shared slot/segment schedule.

    Slots for cell (b, q) = max edge count over the 8 cores (exact, no
    rounding). Batches of 128 slots span blocks freely; each (batch, block)
    intersection is a SEGMENT with its own global index s. Every segment
    runs as a full-128-partition matmul whose one-hot is masked (val=0)
    outside the segment's slots, so the PE config never changes.

    Returns (quota, slot0, runs, segs, seg_map, first, last, T, S):
      runs     [(si, q, t0, R)]
      segs     per batch t: [(b, s)] in emission order
      seg_map  [T, nb] -> s (or -1)
      first/last  {b: s} accumulation start/stop segment
    """
    nb, nq = counts.shape[1], counts.shape[2]
    quota = counts.max(axis=0).astype(np.int64)  # [nb, nq]
    for b in range(nb):
        if quota[b].sum() == 0:
            quota[b, 0] = 1
    sbs = [(u, min(u + sblk, nb)) for u in range(0, nb, sblk)]
    slot0 = np.full((nb, nq), -1, dtype=np.int64)
    runs = []
    spans = []  # per batch t: list of blocks b touching it
    T = 0
    for si, (u0, u1) in enumerate(sbs):
        for q in range(nq):
            tot = int(quota[u0:u1, q].sum())
            if tot == 0:
                continue
            R = -(-tot // P)
            t0 = T
            off = t0 * P
            for b in range(u0, u1):
                if quota[b, q] == 0:
                    continue
                slot0[b, q] = off
                s, e = off, off + int(quota[b, q])
                for k in range(s // P, (e - 1) // P + 1):
                    while len(spans) <= k:
                        spans.append([])
                    spans[k].append(b)
                off = e
            runs.append((si, q, t0, R))
            T += R
            while len(spans) < T:
                spans.append([])
    segs = []
    seg_map = np.full((T, nb), -1, dtype=np.int64)
    S = 0
    first, last = {}, {}
    for t in range(T):
        row = []
        for b in spans[t]:
            seg_map[t, b] = S
            row.append((b, S))
            if b not in first:
                first[b] = S
            last[b] = S
            S += 1
        segs.append(row)
    return quota, slot0, runs, segs, seg_map, first, last, T, S


def _prep(A_rows, A_cols, A_vals, n_cores, npc, ch, sblk):
    nb = (npc + P - 1) // P
    nq = len(ch) - 1
    core = A_rows // npc
    rl = A_rows - core * npc
    blk = rl // P
    dloc = rl % P
    q = np.searchsorted(ch, A_cols, side="right") - 1
    cell = (core * nb + blk) * nq + q
    counts = np.bincount(cell, minlength=n_cores * nb * nq).reshape(
        n_cores, nb, nq
    )
    quota, slot0, runs, segs, seg_map, first, last, T, S = _schedule(
        counts, sblk
    )
    slot0_flat = slot0.reshape(-1)
    metas = []
    for c in range(n_cores):
        m = core == c
        dloc_c, cols_c, vals_c, blk_c = dloc[m], A_cols[m], A_vals[m], blk[m]
        key = blk_c * nq + q[m]
        order = np.argsort(key, kind="stable")
        dloc_c, cols_c, vals_c, key, blk_c = (
            dloc_c[order],
            cols_c[order],
            vals_c[order],
            key[order],
            blk_c[order],
        )
        ccounts = counts[c].reshape(-1)
        starts = np.concatenate([[0], np.cumsum(ccounts)])[:-1]
        pos = np.arange(key.size) - starts[key]
        slot = slot0_flat[key] + pos
        assert (slot >= 0).all()
        t_of = slot // P
        i_of = slot % P
        s_of = seg_map[t_of, blk_c]
        assert (s_of >= 0).all()
        idx16 = (cols_c - ch[q[m][order]]).astype(np.int16)
        idx_flat = np.zeros((16, 8 * T), np.int16)
        idx_flat[i_of % 16, t_of * 8 + i_of // 16] = idx16
        idx_all = np.tile(idx_flat, (8, 1))
        dloc_t = np.zeros((P, S), np.float32)
        val_t = np.zeros((P, S), np.float32)
        dloc_t[i_of, s_of] = dloc_c.astype(np.float32)
        val_t[i_of, s_of] = vals_c
        metas.append((idx_all, dloc_t, val_t))
    return metas, (quota, slot0, runs, segs, first, last, T, S), nb, nq


def _build_program(
    n_nodes, ch, sched, nb, sblk, reps=1,
    do_gather=True, do_oh=True, do_mm=True, do_proj=True,
):
    quota, slot0, runs, segs, first, last, T, S = sched
    if not do_mm:
        do_proj = False  # proj reads the PSUM accumulators mm writes
    sbs = [(u, min(u + sblk, nb)) for u in range(0, nb, sblk)]
    nc = bacc.Bacc(
        "TRN2", target_bir_lowering=False, debug=False, num_devices=N_CORES,
        num_swdge_queues=4, dynamic_dma_scratch_size=SCRATCH,
    )
    x_d = nc.dram_tensor("X", [n_nodes, D], XDT, kind="ExternalInput").ap()
    idx_d = nc.dram_tensor("idx", [P, 8 * T], I16, kind="ExternalInput").ap()
    dloc_d = nc.dram_tensor("dloc", [P, S], F32, kind="ExternalInput").ap()
    val_d = nc.dram_tensor("val", [P, S], F32, kind="ExternalInput").ap()
    wt_d = nc.dram_tensor("wt", [P, D], BF16, kind="ExternalInput").ap()
    bb_d = nc.dram_tensor("bb", [P, D], F32, kind="ExternalInput").ap()
    iota_d = nc.dram_tensor("iota", [P, P], BF16, kind="ExternalInput").ap()
    y_d = nc.dram_tensor("y", [nb * P, D], F32, kind="ExternalOutput").ap()

    qrr = 0  # SWDGE queue round-robin
    with tile.TileContext(nc) as tc:
        with (
            tc.tile_pool(name="const", bufs=1) as cpool,
            tc.tile_pool(name="xg", bufs=3) as xgpool,
            tc.tile_pool(name="oh", bufs=16) as ohpool,
            tc.tile_pool(name="hts", bufs=4) as htspool,
            tc.tile_pool(name="yst", bufs=2) as ystpool,
            tc.tile_pool(name="psh", bufs=6, space="PSUM") as phpool,
            tc.tile_pool(name="psy", bufs=2, space="PSUM") as pypool,
        ):
            idx_s = cpool.tile([P, 8 * T], I16)
            nc.sync.dma_start(out=idx_s[:], in_=idx_d[:])
            dloc_s = cpool.tile([P, S], F32)
            nc.sync.dma_start(out=dloc_s[:], in_=dloc_d[:])
            val_s = cpool.tile([P, S], F32)
            nc.sync.dma_start(out=val_s[:], in_=val_d[:])
            wt_s = cpool.tile([P, D], BF16)
            nc.sync.dma_start(out=wt_s[:], in_=wt_d[:])
            bb_s = cpool.tile([P, D], F32)
            nc.sync.dma_start(out=bb_s[:], in_=bb_d[:])
            iota_s = cpool.tile([P, P], BF16)
            nc.sync.dma_start(out=iota_s[:], in_=iota_d[:])

            for rep in range(reps):
                hp = {}  # (si, gi) -> PSUM tile holding 4 block columns
                ystage = None
                cur_si = -1
                for si, q, t0, R in runs:
                    u0, u1 = sbs[si]
                    if si != cur_si:
                        if cur_si >= 0 and do_proj:
                            pu0, pu1 = sbs[cur_si]
                            g = pu1 - pu0
                            rows = y_d[pu0 * P : pu1 * P, :]
                            nc.sync.dma_start(
                                out=rows.rearrange("(g p) f -> p g f", p=P),
                                in_=ystage[:, : g * D].rearrange(
                                    "p (g f) -> p g f", f=D
                                ),
                            )
                        cur_si = si
                        if do_proj:
                            ystage = ystpool.tile(
                                [P, (u1 - u0) * D], F32, tag="yst"
                            )
                    xg = None
                    if do_gather:
                        xg = xgpool.tile([P, R * D], XDT, tag="xg")
                        nparts = min(4, R)
                        part = -(-R // nparts)
                        for pi in range(nparts):
                            j0 = pi * part
                            j1 = min(R, j0 + part)
                            if j0 >= j1:
                                continue
                            nc.gpsimd.dma_gather(
                                out_ap=xg[:, j0 * D : j1 * D].rearrange(
                                    "p (g f) -> p g f", f=D
                                ),
                                in_ap=x_d[int(ch[q]) : int(ch[q + 1]), :],
                                idxs_ap=idx_s[:, (t0 + j0) * 8 : (t0 + j1) * 8],
                                num_idxs=(j1 - j0) * P,
                                num_idxs_reg=(j1 - j0) * P,
                                elem_size=D,
                                single_packet=SINGLE_PACKET,
                                queue_num=pi,
                            )
                    for k in range(R):
                        t = t0 + k
                        for b, s in segs[t]:
                            gi = b - u0
                            ci = 0
                            oh = None
                            if do_oh:
                                oh = ohpool.tile([P, D], BF16, tag="oh")
                                nc.vector.tensor_scalar(
                                    out=oh[:],
                                    in0=iota_s[:],
                                    scalar1=dloc_s[:, s : s + 1],
                                    scalar2=val_s[:, s : s + 1],
                                    op0=mybir.AluOpType.is_equal,
                                    op1=mybir.AluOpType.mult,
                                )
                            if first[b] == s and do_mm:
                                if (si, gi) not in hp:
                                    hp[(si, gi)] = phpool.tile(
                                        [P, D], F32, tag="hp",
                                        name=f"hp{rep}_{si}_{gi}",
                                    )
                            if do_mm:
                                nc.tensor.matmul(
                                    out=hp[(si, gi)][:, ci * D : (ci + 1) * D],
                                    lhsT=xg[:, k * D : (k + 1) * D]
                                    if do_gather else iota_s[:],
                                    rhs=oh[:] if do_oh else iota_s[:],
                                    start=(first[b] == s),
                                    stop=(last[b] == s),
                                )
                            if last[b] == s and do_proj:
                                hts = htspool.tile([P, D], BF16, tag="hts")
                                nc.scalar.activation(
                                    out=hts[:],
                                    in_=hp[(si, gi)][:, ci * D : (ci + 1) * D],
                                    func=mybir.ActivationFunctionType.Copy,
                                )
                                yps = pypool.tile([P, D], F32, tag="yp")
                                nc.tensor.matmul(
                                    out=yps[:],
                                    lhsT=hts[:],
                                    rhs=wt_s[:],
                                    start=True,
                                    stop=True,
                                )
                                g = b - u0
                                nc.vector.tensor_tensor(
                                    out=ystage[:, g * D : (g + 1) * D],
                                    in0=yps[:],
                                    in1=bb_s[:],
                                    op=mybir.AluOpType.add,
                                )
                if do_proj:
                    pu0, pu1 = sbs[cur_si]
                    g = pu1 - pu0
                    rows = y_d[pu0 * P : pu1 * P, :]
                    nc.sync.dma_start(
                        out=rows.rearrange("(g p) f -> p g f", p=P),
                        in_=ystage[:, : g * D].rearrange("p (g f) -> p g f", f=D),
                    )
    nc.finalize()
    return nc


def _make_in_maps(inputs, n_cores=N_CORES, npc=NPC, chunk=CHUNK, sblk=SBLK):
    X = np.asarray(inputs["X"], dtype=np.float32)
    A_rows = np.asarray(inputs["A_rows"], dtype=np.int64)
    A_cols = np.asarray(inputs["A_cols"], dtype=np.int64)
    A_vals = np.asarray(inputs["A_vals"], dtype=np.float32)
    W = np.asarray(inputs["W"], dtype=np.float32)
    bias = np.asarray(inputs["b"], dtype=np.float32)

    n_nodes = X.shape[0]
    ch = _chunk_bounds(n_nodes, chunk)
    metas, sched, nb, _ = _prep(A_rows, A_cols, A_vals, n_cores, npc, ch, sblk)
    xbf = np.ascontiguousarray(X.astype(mybir.dt.np(XDT)))
    wt = np.ascontiguousarray(W.T.astype(NP_BF16))
    bb = np.broadcast_to(bias[None, :], (P, D)).astype(np.float32).copy()
    iota = np.broadcast_to(
        np.arange(P, dtype=np.float32)[None, :], (P, P)
    ).astype(NP_BF16).copy()
    in_maps = []
    for idx_all, dloc_t, val_t in metas:
        in_maps.append(
            {
                "X": xbf,
                "idx": idx_all,
                "dloc": dloc_t,
                "val": val_t,
                "wt": wt,
                "bb": bb,
                "iota": iota,
            }
        )
    return in_maps, ch, sched, nb


def _run(inputs, trace=False, **kw):
    in_maps, ch, sched, nb = _make_in_maps(inputs)
    nc = _build_program(np.asarray(inputs["X"]).shape[0], ch, sched, nb, SBLK)
    res = run_bass_kernel_spmd(nc, in_maps, list(range(N_CORES)), trace=trace, **kw)
    out = np.concatenate([res.results[c]["y"][:NPC] for c in range(N_CORES)], axis=0)
    return out, res


def kernel(**inputs):
    return _run(inputs, trace=False)[0]
